# revision 3
# baseline (speedup 1.0000x reference)
"""MinGRU (B=4, T=4096, D=1024) TRN2 kernel, 8-core SPMD — tuned PE path.

Sharding: core i handles (batch b = i//2, output-channel half j = i%2).

Knobs vs baseline:
  MDT="bf16"    matmul operands in bf16 (host-cast): FWL halves LDWEIGHTS,
                x DMA traffic halves; rel err ~2e-3 (gate is 2e-2).
  KSPLIT=4     K=128 contraction split into 4x32-row subarray matmuls via
                tile_position so LDWEIGHTS overlaps other subarrays' streams.
  INTERLEAVE   alternate z/h accumulation chains per k (PSUM bank alternation).
"""

import numpy as np

MDT = "bf16"        # "f32r" | "bf16"
KSPLIT = 1          # 1 | 2 | 4
INTERLEAVE = False

_B, _T, _D = 4, 4096, 1024
_EH = 512
_NG = _EH // 128
_TT = 512
_NT = _T // _TT
_NK = _D // 128


def _build(reps=1, loop_n=None):
    from contextlib import ExitStack
    from concourse import bacc, mybir, tile

    f32 = mybir.dt.float32
    mdt = mybir.dt.float32r if MDT == "f32r" else mybir.dt.bfloat16
    AF = mybir.ActivationFunctionType
    OP = mybir.AluOpType
    RS = 128 // KSPLIT

    nc = bacc.Bacc("TRN2", debug=False, num_devices=8)
    xt = nc.dram_tensor("xt", [_D, _T], mdt, kind="ExternalInput").ap()
    wzt = nc.dram_tensor("wzt", [_D, _EH], mdt, kind="ExternalInput").ap()
    wht = nc.dram_tensor("wht", [_D, _EH], mdt, kind="ExternalInput").ap()
    bzt = nc.dram_tensor("bzt", [128, _NG], f32, kind="ExternalInput").ap()
    bht = nc.dram_tensor("bht", [128, _NG], f32, kind="ExternalInput").ap()
    hout = nc.dram_tensor("h", [_EH, _T], f32, kind="ExternalOutput").ap()

    with tile.TileContext(nc) as tc, ExitStack() as ctx:
        wpool = ctx.enter_context(tc.tile_pool(name="w", bufs=1))
        xpool = ctx.enter_context(tc.tile_pool(name="x", bufs=3))
        vpool = ctx.enter_context(tc.tile_pool(name="v", bufs=3))
        hpool = ctx.enter_context(tc.tile_pool(name="h", bufs=2))
        ppool = ctx.enter_context(tc.tile_pool(name="p", bufs=4, space="PSUM"))

        def load_x(t, n_chunks=2):
            xs = xpool.tile([128, _NK * _TT], mdt, tag="x", name="xs")
            step = _NK // n_chunks
            for c in range(n_chunks):
                ks = c * step
                nc.sync.dma_start(
                    xs[:, ks * _TT:(ks + step) * _TT].rearrange(
                        "p (k t) -> p k t", k=step),
                    xt.rearrange("(k p) t -> p k t", p=128)[
                        :, ks:ks + step, t * _TT:(t + 1) * _TT],
                )
            return xs

        xs0 = xpool.tile([128, _NK * _TT], mdt, tag="x", name="xs0")
        wz_sb = wpool.tile([128, _NK * _EH], mdt, tag="wz", name="wz_sb")
        wh_sb = wpool.tile([128, _NK * _EH], mdt, tag="wh", name="wh_sb")
        bz_sb = wpool.tile([128, _NG], f32, tag="bz", name="bz_sb")
        bh_sb = wpool.tile([128, _NG], f32, tag="bh", name="bh_sb")

        def x0_chunk(ks, nk):
            nc.sync.dma_start(
                xs0[:, ks * _TT:(ks + nk) * _TT].rearrange(
                    "p (k t) -> p k t", k=nk),
                xt.rearrange("(k p) t -> p k t", p=128)[
                    :, ks:ks + nk, 0:_TT],
            )

        def w_chunk(k):
            nc.sync.dma_start(
                wz_sb[:, k * _EH:(k + 1) * _EH],
                wzt[k * 128:(k + 1) * 128, :],
            )
            nc.sync.dma_start(
                wh_sb[:, k * _EH:(k + 1) * _EH],
                wht[k * 128:(k + 1) * 128, :],
            )

        x0_chunk(0, 2)
        w_chunk(0)
        w_chunk(1)
        nc.sync.dma_start(bz_sb[:], bzt)
        nc.sync.dma_start(bh_sb[:], bht)
        x0_chunk(2, 2)
        w_chunk(2)
        w_chunk(3)
        x0_chunk(4, 2)
        w_chunk(4)
        w_chunk(5)
        x0_chunk(6, 2)
        w_chunk(6)
        w_chunk(7)

        def mm_group(pp, w_sb_, g, xs, c0, w, k, first, last_k):
            # one K=128 tile as KSPLIT subarray matmuls
            base = k * _EH + g * 128
            for rg in range(KSPLIT):
                nc.tensor.matmul(
                    pp[:],
                    lhsT=w_sb_[rg * RS:(rg + 1) * RS, base:base + 128],
                    rhs=xs[rg * RS:(rg + 1) * RS,
                           k * _TT + c0: k * _TT + c0 + w],
                    start=(first and rg == 0),
                    stop=(last_k and rg == KSPLIT - 1),
                    tile_position=(rg * RS, 0) if KSPLIT > 1 else None,
                )

        def body(first):
          hprev = [None] * _NG
          xs_cur = xs0 if first else load_x(0)
          for t in range(_NT):
            xs = xs_cur
            if t + 1 < _NT:
                xs_cur = load_x(t + 1)   # prefetch next t-tile
            for g in range(_NG):
                last = (t == _NT - 1 and g == _NG - 1)
                halves = ((0, _TT),)
                prev_ap = None if t == 0 else hprev[g][:, _TT - 1:_TT]
                for (c0, w) in halves:
                    pz = ppool.tile([128, w], f32, tag="pz", name="pz")
                    ph = ppool.tile([128, w], f32, tag="ph", name="ph")
                    if INTERLEAVE:
                        for k in range(_NK):
                            mm_group(pz, wz_sb, g, xs, c0, w, k,
                                     k == 0, k == _NK - 1)
                            mm_group(ph, wh_sb, g, xs, c0, w, k,
                                     k == 0, k == _NK - 1)
                    else:
                        for k in range(_NK):
                            mm_group(pz, wz_sb, g, xs, c0, w, k,
                                     k == 0, k == _NK - 1)
                        for k in range(_NK):
                            mm_group(ph, wh_sb, g, xs, c0, w, k,
                                     k == 0, k == _NK - 1)
                    z = vpool.tile([128, w], f32, tag="z", name="z")
                    nc.scalar.activation(z[:], pz[:], AF.Sigmoid,
                                         bias=bz_sb[:, g:g + 1])
                    av = vpool.tile([128, w], f32, tag="a", name="av")
                    nc.gpsimd.tensor_scalar(av[:], z[:], -1.0, 1.0,
                                            OP.mult, OP.add)
                    bv = vpool.tile([128, w], f32, tag="b", name="bv")
                    nc.vector.scalar_tensor_tensor(
                        bv[:], ph[:], bh_sb[:, g:g + 1], z[:], OP.add, OP.mult
                    )
                    hb = hpool.tile([128, w], f32, tag=f"h{g}", name="hb")
                    init = 0.0 if prev_ap is None else prev_ap
                    nc.vector.tensor_tensor_scan(hb[:], av[:], bv[:], init,
                                                 OP.mult, OP.add)
                    prev_ap = hb[:, w - 1:w]
                    if not last:
                        hprev[g] = hb
                    nc.sync.dma_start(
                        hout[g * 128:(g + 1) * 128,
                             t * _TT + c0: t * _TT + c0 + w], hb[:]
                    )

        if loop_n is not None:
            body(True)
            from concourse import mybir as _mb
            with tc.For_i(0, loop_n, 1, hint_engines=(
                    _mb.EngineType.PE, _mb.EngineType.SP,
                    _mb.EngineType.DVE, _mb.EngineType.Activation,
                    _mb.EngineType.Pool)):
                body(False)
        else:
            for rep in range(reps):
                body(rep == 0)
    nc.compile()
    return nc


_NC_CACHE = None


def _shard_inputs(inputs):
    import ml_dtypes
    npdt = np.float32 if MDT == "f32r" else ml_dtypes.bfloat16

    x = np.asarray(inputs["x"], dtype=np.float32)
    Wz = np.asarray(inputs["Wz"], dtype=np.float32)
    bz = np.asarray(inputs["bz"], dtype=np.float32)
    Wh = np.asarray(inputs["Wh"], dtype=np.float32)
    bh = np.asarray(inputs["bh"], dtype=np.float32)

    wzT = np.ascontiguousarray(Wz.T).astype(npdt)
    whT = np.ascontiguousarray(Wh.T).astype(npdt)

    in_maps = []
    for i in range(8):
        b, j = i // 2, i % 2
        sl = slice(j * _EH, (j + 1) * _EH)
        in_maps.append({
            "xt": np.ascontiguousarray(x[b].T).astype(npdt),
            "wzt": np.ascontiguousarray(wzT[:, sl]),
            "wht": np.ascontiguousarray(whT[:, sl]),
            "bzt": np.ascontiguousarray(bz[sl].reshape(_NG, 128).T),
            "bht": np.ascontiguousarray(bh[sl].reshape(_NG, 128).T),
        })
    return in_maps


def run(inputs, trace=False, tmpdir=None):
    global _NC_CACHE
    from concourse.bass_utils import run_bass_kernel_spmd

    if _NC_CACHE is None:
        _NC_CACHE = _build()
    nc = _NC_CACHE

    in_maps = _shard_inputs(inputs)

    res = run_bass_kernel_spmd(
        nc, in_maps, core_ids=list(range(8)), trace=trace, tmpdir=tmpdir
    )

    out = np.empty((_B, _T, _D), dtype=np.float32)
    for i in range(8):
        b, j = i // 2, i % 2
        out[b, :, j * _EH:(j + 1) * _EH] = res.results[i]["h"].T
    return out, res


def kernel(**inputs):
    out, _ = run(inputs, trace=False)
    return out
